# revision 1
# baseline (speedup 1.0000x reference)
"""Trainium2 Bass kernel for nn_AlternateAttention (3-block transformer:
global attention, lower-band attention, upper-band attention, each with MLP).

Sharding: 8 cores = 2 batches x 4 sequence chunks of 1024 tokens, each core
processing an extended window of 1152 tokens (64-token halo each side) so the
banded blocks need no inter-core communication. Block 0 (global attention)
needs full-sequence K/V; each core recomputes them from the replicated raw
input (LN1+KV projection over the full 4096 tokens of its batch).

Layout: activations live TRANSPOSED in SBUF ([feature, token]) so projections
are natural matmuls (lhsT = W^T chunks). LN stats are computed with ones-
matmul partition reductions; per-token mean/rstd are broadcast with K=1
matmuls. Softmax is computed without max subtraction (scores are O(1) by
construction), in the scores^T layout, with row-of-ones appended to V to get
the normalizer for free.
"""
import contextlib
import numpy as np
import ml_dtypes

# ---- problem constants (hardcoded per contract) ----
B, S, E, H, W_BAND, FF = 2, 4096, 512, 8, 16, 2048
HD = E // H                      # 64
N_CORES = 8
CHUNKS = 4                       # sequence chunks per batch
OWN = S // CHUNKS                # 1024
HALO = 64
T = OWN + 2 * HALO               # 1152 = 9*128
P = 128
NT = 384                         # token tile (3 per T)
NQT = T // NT                    # 3
NKC = S // P                     # 32 k-chunks for global attention
NTC = T // P                     # 9 token chunks of ext window
EC = E // P                      # 4 feature strips
FFC = FF // P                    # 16

EXT_STARTS = [max(0, min(OWN * c - HALO, S - T)) for c in range(CHUNKS)]
OWN_OFF = [OWN * c - EXT_STARTS[c] for c in range(CHUNKS)]

_EXEC_CACHE = {}
_PHASE_MARKS = []
_PHASE_OF = {}

bf16 = ml_dtypes.bfloat16


# ------------------------------------------------------------------
# device program
# ------------------------------------------------------------------
def _patch_act_tables():
    # The bacc table-load placement maps exp -> 'exp_and_others' and
    # ln -> 'natural_log', thrashing the ACT table RAM (~1.3us per switch,
    # dozens of switches). Restrict the choice to the two sets that cover
    # everything this kernel uses so exp/ln never evict each other.
    import concourse.hw_specs as hw_specs
    import concourse.bacc as bacc_mod
    import concourse.bass_interp as bass_interp
    if getattr(hw_specs, "_aa_patched", False):
        return
    orig = hw_specs.get_activation_tables
    keep = {"natural_log_exp_and_others", "gelu_apprx_tanh_and_others"}

    def _gat(arch):
        tabs = orig(arch)
        return {k: (v if k in keep else set()) for k, v in tabs.items()}

    hw_specs.get_activation_tables = _gat
    bacc_mod.get_activation_tables = _gat
    bass_interp.get_activation_tables = _gat
    hw_specs._aa_patched = True


def _build_nc(use_bias: bool, repeat: int = 1):
    import concourse.bacc as bacc
    import concourse.mybir as mybir
    import concourse.tile as tile

    _patch_act_tables()

    f32 = mybir.dt.float32
    b16 = mybir.dt.bfloat16
    AF = mybir.ActivationFunctionType
    OP = mybir.AluOpType

    nc = bacc.Bacc("TRN2", target_bir_lowering=False, debug=False,
                   num_devices=N_CORES)
    _PHASE_MARKS.clear()

    def mark(label):
        n = sum(len(b.instructions) for b in nc.main_func.blocks)
        _PHASE_MARKS.append((label, n))

    # ---- dram tensors ----
    xT = nc.dram_tensor("xT", [E, T], f32, kind="ExternalInput")
    xfT = nc.dram_tensor("xfT", [E, S], b16, kind="ExternalInput")
    wq, wo, w1, w2, bias_d = [], [], [], [], []
    for i in range(3):
        wq.append(nc.dram_tensor(f"qkvT{i}", [EC, P, 3 * E], b16, kind="ExternalInput"))
        wo.append(nc.dram_tensor(f"outT{i}", [EC, P, E], b16, kind="ExternalInput"))
        w1.append(nc.dram_tensor(f"fc1T{i}", [EC, P, FF], b16, kind="ExternalInput"))
        w2.append(nc.dram_tensor(f"fc2T{i}", [FFC, P, E], b16, kind="ExternalInput"))
        if use_bias:
            # packed per-feature biases for transposed-layout outputs:
            # [q(4xP) | k(4xP) | out(4xP) | fc1(16xP) | fc2(4xP)] -> [P, 32]
            bias_d.append(nc.dram_tensor(f"bias{i}", [P, 32], f32, kind="ExternalInput"))
            bias_d.append(nc.dram_tensor(f"vbias{i}", [1, E], b16, kind="ExternalInput"))
    # 0/1 bf16 stripe-mask tiles for banded attention (384-query tiles,
    # 4 relative k-chunks each)
    mask_d = {}
    for bnd in ("lo", "up"):
        for c in range(4):
            nm = f"m_{bnd}{c}"
            mask_d[nm] = nc.dram_tensor(nm, [P, NT], b16, kind="ExternalInput")
    yT = nc.dram_tensor("yT", [E, T], f32, kind="ExternalOutput")

    from concourse.tile import add_dep_helper as _adh

    with tile.TileContext(nc) as tc, contextlib.ExitStack() as ctx:
        pool = lambda name, bufs, **kw: ctx.enter_context(
            tc.tile_pool(name=name, bufs=bufs, **kw))

        # Order ACT instructions across table-set boundaries (exp/ln vs gelu)
        # so the activation-table RAM isn't thrashed (~1.3us per reload).
        # Within a set no ordering is imposed.
        _act_sets = {AF.Gelu_apprx_tanh: 1}
        _prev_set = [None]
        _prev_insts = [[]]
        _cur_insts = [[]]

        def act(out, in_, func, **kw):
            inst = nc.scalar.activation(out, in_, func, **kw)
            if func == AF.Copy:
                return inst
            s = _act_sets.get(func, 0)
            if _prev_set[0] is None:
                _prev_set[0] = s
            if s != _prev_set[0]:
                _prev_insts[0] = _cur_insts[0][-64:]
                _cur_insts[0] = []
                _prev_set[0] = s
            for p in _prev_insts[0]:
                _adh(inst.ins, p.ins, sync=True, reason="act-table batch order")
            _cur_insts[0].append(inst)
            return inst

        # ---- pools live for the whole kernel ----
        p_x = pool("x", 2)           # residual strips f32, tags x0..x3
        p_h = pool("h", 1)           # LN output strips bf16
        p_xb = pool("xb", 2)         # per-nt bf16 casts for stats
        p_sq = pool("sq", 4)         # per-nt squares bf16
        p_qt = pool("qt", 1)         # QT [P, EC, T] bf16
        p_ot = pool("ot", 1)         # attention out strips bf16
        p_wqkv = pool("wqkv", 1)     # qkv weights [P, EC, 3E]
        p_wout = pool("wout", 1)
        p_probs = pool("probs", 4 if not use_bias else 3)   # exp outputs bf16
        p_tmp = pool("tmp", 2)       # f32 [P, NT] temporaries
        p_sm = pool("sm", 2)         # small [1, n] stat vectors
        p_c = pool("const", 1)       # ones, masks, biases
        # PSUM: exactly 8 banks, hand-assigned tags
        p_ps = pool("ps", 1, space="PSUM")
        # sc0-sc2: attention scores (+ LN stat sums share sc0/sc1, rstd
        # broadcast shares sc2 - temporally disjoint from scores)
        # po0/po1: attention output accumulators
        # mm0/mm1: gemm accumulators
        # bc0: mean broadcast / softmax 1/l broadcast

        ones = p_c.tile([P, P], b16)
        nc.vector.memset(ones[:], 1.0)
        eps1 = p_c.tile([1, 1], f32)
        nc.vector.memset(eps1[:], 1e-5)
        masks = {}
        for nm, d in mask_d.items():
            mt = p_c.tile([P, NT], b16, tag=f"mask_{nm}", name=f"mask_{nm}")
            nc.sync.dma_start(mt[:], d[:])
            masks[nm] = mt
        bias_t, vbias_t = [], []
        if use_bias:
            for i in range(3):
                bt = p_c.tile([P, 32], f32, tag=f"bias{i}", name=f"bias{i}")
                nc.sync.dma_start(bt[:], bias_d[2 * i][:])
                bias_t.append(bt)
                vt = p_c.tile([1, E], b16, tag=f"vbias{i}", name=f"vbias{i}")
                nc.sync.dma_start(vt[:], bias_d[2 * i + 1][:])
                vbias_t.append(vt)

        def bslice(i, group, oc):
            base = {"qkv": 0, "out": 8, "fc1": 12, "fc2": 28}[group]
            return bias_t[i][:, base + oc:base + oc + 1]

        def add_vbias(i, ps):
            # V projection output is in normal layout [token, feat]: bias
            # varies along the free dim -> broadcast with a K=1 matmul.
            vb = p_ps.tile([P, 512], f32, tag="bc0", name="vb")
            nc.tensor.matmul(vb[:], ones[0:1, :], vbias_t[i][:],
                             start=True, stop=True)
            vbs = p_tmp.tile([P, 512], f32, tag="vbs", name="vbs")
            nc.vector.tensor_copy(vbs[:], vb[:])
            nc.vector.tensor_add(ps[:], ps[:], vbs[:])

        def psum_to_sbuf(dst_ap, ps_ap, i, group, oc, eng="dve"):
            """copy matmul accumulator to sbuf, adding bias if enabled"""
            if eng == "act" and not use_bias:
                act(dst_ap, ps_ap, AF.Copy)
            elif use_bias:
                nc.vector.tensor_scalar(dst_ap, ps_ap, bslice(i, group, oc),
                                        None, OP.add)
            else:
                nc.vector.tensor_copy(dst_ap, ps_ap)

        def _emit_once():
            # load residual strips
            xs = []
            for s in range(EC):
                t = p_x.tile([P, T], f32, tag=f"x{s}", name=f"xin{s}")
                nc.sync.dma_start(t[:], xT[P * s:P * (s + 1), :])
                xs.append(t)

            # ---------- layernorm ----------
            def layernorm(x_strips, Tn, in_f32, htag):
                """x_strips: 4 strips [P, Tn] (f32 or bf16) -> 4 bf16 strips"""
                hs = [p_h.tile([P, Tn], b16, tag=f"{htag}{s}", name=f"{htag}{s}")
                      for s in range(EC)]
                step = 512 if Tn % 512 == 0 else NT
                nss = [(k * step, min(step, Tn - k * step))
                       for k in range((Tn + step - 1) // step)]
                for (o, n) in nss:
                    sl = slice(o, o + n)
                    s1 = p_ps.tile([1, 512], f32, tag="sc0", name="s1")
                    s2 = p_ps.tile([1, 512], f32, tag="sc1", name="s2")
                    xb_nts = {}
                    for s in range(EC):
                        if in_f32:
                            xb_nt = p_xb.tile([P, 512], b16, tag=f"xbn{s}",
                                              name="xbn")
                            nc.vector.tensor_copy(xb_nt[:, :n], x_strips[s][:, sl])
                            rhs_x = xb_nt[:, :n]
                            xb_nts[s] = rhs_x
                        else:
                            rhs_x = x_strips[s][:, sl]
                        nc.tensor.matmul(s1[:, :n], ones[:, 0:1], rhs_x,
                                         start=(s == 0), stop=(s == EC - 1))
                        sq_nt = p_sq.tile([P, 512], b16, tag="sqn", name="sqn")
                        nc.vector.tensor_mul(sq_nt[:, :n], rhs_x, rhs_x)
                        nc.tensor.matmul(s2[:, :n], ones[:, 0:1], sq_nt[:, :n],
                                         start=(s == 0), stop=(s == EC - 1))
                    m_b = p_sm.tile([1, 512], b16, tag="m_b", name="m_b")
                    nc.vector.tensor_scalar(m_b[:, :n], s1[:, :n], 1.0 / E, None, OP.mult)
                    stt = p_sm.tile([1, 1024], f32, tag="stt", name="stt")
                    sa, sb = stt[:, 0:n], stt[:, 512:512 + n]
                    nc.vector.tensor_scalar(sa, s2[:, :n], 1.0 / E, None, OP.mult)
                    nc.vector.tensor_mul(sb, m_b[:, :n], m_b[:, :n])
                    nc.vector.tensor_sub(sa, sa, sb)
                    act(sb, sa, AF.Ln, bias=eps1[:])
                    lnv = sb
                    r_b = p_sm.tile([1, 512], b16, tag="r_b", name="r_b")
                    act(r_b[:, :n], lnv, AF.Exp, scale=-0.5)
                    mB = p_ps.tile([P, 512], f32, tag="bc0", name="mB")
                    nc.tensor.matmul(mB[:, :n], ones[0:1, :], m_b[:, :n],
                                     start=True, stop=True)
                    rB = p_ps.tile([P, 512], f32, tag="sc2", name="rB")
                    nc.tensor.matmul(rB[:, :n], ones[0:1, :], r_b[:, :n],
                                     start=True, stop=True)
                    mBs = p_tmp.tile([P, 512], b16, tag="mBs", name="mBs")
                    act(mBs[:, :n], mB[:, :n], AF.Copy)
                    rBs = p_tmp.tile([P, 512], b16, tag="rBs", name="rBs")
                    act(rBs[:, :n], rB[:, :n], AF.Copy)
                    for s in range(EC):
                        if in_f32:
                            xbs = xb_nts[s]
                        else:
                            xbs = x_strips[s][:, sl]
                        t0 = p_tmp.tile([P, 512], b16, tag="lnt", name="lnt")
                        nc.vector.tensor_sub(t0[:, :n], xbs, mBs[:, :n])
                        nc.vector.tensor_mul(hs[s][:, sl], t0[:, :n], rBs[:, :n])
                return hs

            # ---------- transposed GEMM ----------
            _gm_cycle = [0]
            _sc_cycle = [0]
            _po_cycle = [0]

            def ps_tile(cycle=False):
                t = ("mm0", "mm1", "po0", "po1")[_gm_cycle[0] % 4]
                _gm_cycle[0] += 1
                return p_ps.tile([P, 512], f32, tag=t, name="gps")

            def sc_tile(tags=("sc0", "sc1", "sc2")):
                t = tags[_sc_cycle[0] % len(tags)]
                _sc_cycle[0] += 1
                return p_ps.tile([P, 512], f32, tag=t, name="sps")

            def po_tile(tags=("po0", "po1")):
                t = tags[_po_cycle[0] % len(tags)]
                _po_cycle[0] += 1
                return p_ps.tile([HD + 1, 512], f32, tag=t, name="po")

            def gemm(w_tile, col0, n_oc, x_strips, Tn, post, n_ec=EC):
                """out^T[oc] = sum_ec w_tile[:,ec,col0+oc*P:...].T @ x_strips[ec]"""
                step = 512 if Tn % 512 == 0 else NT
                nss = [(k * step, min(step, Tn - k * step))
                       for k in range((Tn + step - 1) // step)]
                for (o, n) in nss:
                    for oc in range(n_oc):
                        ps = ps_tile()
                        for ec in range(n_ec):
                            nc.tensor.matmul(
                                ps[:, :n],
                                w_tile[:, ec, col0 + oc * P:col0 + (oc + 1) * P],
                                x_strips[ec][:, o:o + n],
                                start=(ec == 0), stop=(ec == n_ec - 1))
                        post(oc, o, n, ps)

            # ---------- attention core (shared) ----------
            def attn_head_qt(kt_tile, q_tile, v_tile, h_, qt0, qn, kcs, mask_for,
                             ot_strips, windows=None):
                """one (head, query-tile): scores^T -> exp -> (mask) -> AV -> scale"""
                hp, hh = h_ // 2, h_ % 2
                banded = windows is not None
                po = po_tile(("po0", "po1", "sc2") if banded else ("po0", "po1"))
                first = True
                n_kc = len(kcs)
                covered = []  # disjoint sorted [lo, hi) q-ranges with a start=True writer
                for idx, (kc, mk) in enumerate(zip(kcs, mask_for)):
                    qo, qw = windows[idx] if windows is not None else (0, qn)
                    sps = sc_tile(("sc0", "sc1") if banded else ("sc0", "sc1", "sc2"))
                    nc.tensor.matmul(
                        sps[:, :qw],
                        kt_tile[HD * hh:HD * (hh + 1), hp, kc * P:(kc + 1) * P],
                        q_tile[HD * hh:HD * (hh + 1), hp,
                               qt0 + qo:qt0 + qo + qw],
                        start=True, stop=True)
                    pr = p_probs.tile([P, 512], b16, tag="pr", name="pr")
                    act(pr[:, :qw], sps[:, :qw], AF.Exp, scale=0.125)
                    if mk is not None:
                        prm = p_probs.tile([P, 512], b16, tag="prm", name="prm")
                        nc.vector.tensor_mul(prm[:, :qw], pr[:, :qw],
                                             masks[mk][:, qo:qo + qw])
                        pr = prm
                    if windows is None:
                        nc.tensor.matmul(po[:, :qn], v_tile[:, kc, h_, :],
                                         pr[:, :qn],
                                         start=first, stop=(idx == n_kc - 1))
                    else:
                        # split the AV into start=True parts (first writer of
                        # those q-columns; PSUM has_written is per element) and
                        # accumulate parts over already-written columns
                        parts = []
                        pos = qo
                        for (clo, chi) in covered + [(qo + qw, qo + qw)]:
                            if pos >= qo + qw:
                                break
                            if chi <= pos:
                                continue
                            if clo > pos:
                                parts.append((pos, min(clo, qo + qw), True))
                            if clo < qo + qw:
                                lo = max(clo, pos)
                                hi = min(chi, qo + qw)
                                if lo < hi:
                                    parts.append((lo, hi, False))
                            pos = max(pos, chi)
                        for (lo, hi, is_new) in parts:
                            nc.tensor.matmul(
                                po[:, lo:hi], v_tile[:, kc, h_, :],
                                pr[:, lo - qo:hi - qo],
                                start=is_new, stop=False,
                                skip_group_check=True)
                        covered.append((qo, qo + qw))
                        covered = sorted(covered)
                        merged = []
                        for (lo, hi) in covered:
                            if merged and lo <= merged[-1][1]:
                                merged[-1] = (merged[-1][0], max(hi, merged[-1][1]))
                            else:
                                merged.append((lo, hi))
                        covered = merged
                    first = False
                ou = p_tmp.tile([HD + 1, 512], b16, tag="ou", name="ou")
                with nc.allow_low_precision(reason="bf16 softmax normalizer"):
                    nc.vector.tensor_copy(ou[:, :qn], po[:, :qn])  # frees po bank
                    linv = p_sm.tile([1, 512], b16, tag="linv", name="linv")
                    nc.vector.reciprocal(linv[:, :qn], ou[HD:HD + 1, :qn])
                lB = p_ps.tile([P, 512], f32, tag="bc0", name="lB")
                nc.tensor.matmul(lB[:HD, :qn], ones[0:1, :HD], linv[:, :qn],
                                 start=True, stop=True)
                nc.vector.tensor_mul(
                    ot_strips[hp][HD * hh:HD * (hh + 1), qt0:qt0 + qn],
                    ou[:HD, :qn], lB[:HD, :qn])

            # ==================================================================
            # BLOCK 0: global attention
            # ==================================================================
            wqkv = p_wqkv.tile([P, EC, 3 * E], b16, tag="wqkv", name="wqkv0")
            for ec in range(EC):
                nc.sync.dma_start(wqkv[:, ec, :], wq[0][ec])
            wout = p_wout.tile([P, EC, E], b16, tag="wout", name="wout0")
            for ec in range(EC):
                nc.sync.dma_start(wout[:, ec, :], wo[0][ec])

            mark("b0.ln1+q")
            h1 = layernorm(xs, T, True, "h")
            # Q projection (ext window)
            qt_t = p_qt.tile([P, EC, T], b16, tag="qt", name="qt0")
            gemm(wqkv, 0, EC, h1, T,
                 lambda oc, o, n, ps: psum_to_sbuf(qt_t[:, oc, o:o + n], ps[:, :n],
                                                   0, "qkv", oc))

            mark("b0.kv")
            ot0 = p_ot.tile([P, EC, T], b16, tag="ot", name="ot0")
            ot_strips = [ot0[:, s, :] for s in range(EC)]
            with tc.tile_pool(name="kvfull", bufs=1) as p_kv, \
                 tc.tile_pool(name="xpan", bufs=3 if not use_bias else 1) as p_xp:
                ktf = p_kv.tile([P, EC, S], b16, tag="ktf", name="ktf")
                vf = p_kv.tile([P, NKC, H, HD + 1], b16, tag="vf", name="vf")
                nc.vector.memset(vf[:, :, :, HD:HD + 1], 1.0)
                for pan in range(S // 512):
                    xp = p_xp.tile([P, EC, 512], b16, tag="xp", name="xp")
                    for s in range(EC):
                        nc.sync.dma_start(xp[:, s, :],
                                          xfT[P * s:P * (s + 1), 512 * pan:512 * (pan + 1)])
                    hp_ = layernorm([xp[:, s, :] for s in range(EC)], 512, False, "hp")
                    # K^T columns for this panel
                    for oc in range(EC):
                        ps = ps_tile()
                        for ec in range(EC):
                            nc.tensor.matmul(ps[:],
                                             wqkv[:, ec, E + oc * P:E + (oc + 1) * P],
                                             hp_[ec][:],
                                             start=(ec == 0), stop=(ec == EC - 1))
                        psum_to_sbuf(ktf[:, oc, 512 * pan:512 * (pan + 1)], ps[:],
                                     0, "qkv", EC + oc, eng="act")
                    # V (normal layout) for this panel
                    for tck in range(4):
                        ps = ps_tile()
                        for ec in range(EC):
                            nc.tensor.matmul(ps[:],
                                             hp_[ec][:, tck * P:(tck + 1) * P],
                                             wqkv[:, ec, 2 * E:3 * E],
                                             start=(ec == 0), stop=(ec == EC - 1))
                        kc = pan * 4 + tck
                        if use_bias:
                            add_vbias(0, ps)
                        act(vf[:, kc, :, 0:HD],
                            ps[:].rearrange("p (h d) -> p h d", h=H), AF.Copy)
                mark("b0.attn")
                # attention (qt outer so out-proj can start per query tile)
                for (q0, qn_) in ((0, 512), (512, 512), (1024, 128)):
                    for h_ in range(H):
                        attn_head_qt(ktf, qt_t, vf, h_, q0, qn_,
                                     list(range(NKC)), [None] * NKC, ot_strips)

            # ---- pools for the post-block0 phases (opened after kvfull frees,
            # closed at end of emission so repeat>1 can reopen) ----
            _lstack = contextlib.ExitStack()
            lpool = lambda name, bufs, **kw: _lstack.enter_context(
                tc.tile_pool(name=name, bufs=bufs, **kw))
            p_kt = lpool("kt", 1)        # KT (banded) [P, EC, T] bf16
            p_v = lpool("v", 1)          # V_ext [P, NTC, H, HD+1] bf16
            p_g = lpool("g", 2 if not use_bias else 1)          # gelu out [P, FFC, NT] bf16
            p_wfc1 = lpool("wfc1", 1)
            p_wfc2 = lpool("wfc2", 1)

            # ---------- MLP (ln2 + fc1 + gelu + fc2 + residual) ----------
            def mlp(i, x_strips):
                h2 = layernorm(x_strips, T, True, "h")
                wf1 = p_wfc1.tile([P, EC, FF], b16, tag="wfc1", name=f"wfc1_{i}")
                for ec in range(EC):
                    nc.sync.dma_start(wf1[:, ec, :], w1[i][ec])
                wf2 = p_wfc2.tile([P, FFC, E], b16, tag="wfc2", name=f"wfc2_{i}")
                for fc in range(FFC):
                    nc.sync.dma_start(wf2[:, fc, :], w2[i][fc])
                x_new = [p_x.tile([P, T], f32, tag=f"x{s}", name=f"xm{i}_{s}")
                         for s in range(EC)]
                for nt in range(NQT):
                    o0 = nt * NT
                    g = p_g.tile([P, FFC, NT], b16, tag="g", name="g")
                    for fc in range(FFC):
                        ps = ps_tile()
                        for ec in range(EC):
                            nc.tensor.matmul(ps[:, :NT],
                                             wf1[:, ec, fc * P:(fc + 1) * P],
                                             h2[ec][:, o0:o0 + NT],
                                             start=(ec == 0), stop=(ec == EC - 1))
                        if use_bias:
                            nc.vector.tensor_scalar(ps[:, :NT], ps[:, :NT],
                                                    bslice(i, "fc1", fc), None, OP.add)
                        act(g[:, fc, :], ps[:, :NT], AF.Gelu_apprx_tanh)
                    for oc in range(EC):
                        ps = ps_tile()
                        for fc in range(FFC):
                            nc.tensor.matmul(ps[:, :NT],
                                             wf2[:, fc, oc * P:(oc + 1) * P],
                                             g[:, fc, :],
                                             start=(fc == 0), stop=(fc == FFC - 1))
                        if use_bias:
                            nc.vector.tensor_scalar(ps[:, :NT], ps[:, :NT],
                                                    bslice(i, "fc2", oc), None, OP.add)
                        nc.vector.tensor_add(x_new[oc][:, o0:o0 + NT],
                                             x_strips[oc][:, o0:o0 + NT], ps[:, :NT])
                return x_new

            mark("b0.proj+mlp")
            # block 0 out projection + residual + MLP
            x1 = [p_x.tile([P, T], f32, tag=f"x{s}", name=f"x1_{s}")
                  for s in range(EC)]
            def post_out0(oc, o, n, ps):
                if use_bias:
                    nc.vector.tensor_scalar(ps[:, :n], ps[:, :n],
                                            bslice(0, "out", oc), None, OP.add)
                nc.vector.tensor_add(x1[oc][:, o:o + n], xs[oc][:, o:o + n], ps[:, :n])
            gemm(wout, 0, EC, ot_strips, T, post_out0)
            x1 = mlp(0, x1)

            # ==================================================================
            # BLOCKS 1, 2: banded attention
            # ==================================================================
            mark("banded")
            x_cur = x1
            for i in (1, 2):
                lower = (i == 1)
                mark(f"b{i}.ln1qkv")
                wqkv = p_wqkv.tile([P, EC, 3 * E], b16, tag="wqkv", name=f"wqkv{i}")
                for ec in range(EC):
                    nc.sync.dma_start(wqkv[:, ec, :], wq[i][ec])
                wout = p_wout.tile([P, EC, E], b16, tag="wout", name=f"wout{i}")
                for ec in range(EC):
                    nc.sync.dma_start(wout[:, ec, :], wo[i][ec])
                h1 = layernorm(x_cur, T, True, "h")
                qt_t = p_qt.tile([P, EC, T], b16, tag="qt", name=f"qt{i}")
                kt_t = p_kt.tile([P, EC, T], b16, tag="kt", name=f"kt{i}")
                v_t = p_v.tile([P, NTC, H, HD + 1], b16, tag="v", name=f"v{i}")
                nc.vector.memset(v_t[:, :, :, HD:HD + 1], 1.0)
                otb = p_ot.tile([P, EC, T], b16, tag="ot", name=f"ot{i}")
                ot_strips = [otb[:, s, :] for s in range(EC)]
                mark(f"b{i}.attn")
                WINS = ([(0, 15), (0, 143), (128, 143), (256, 128)] if lower
                        else [(0, 143), (113, 143), (241, 143), (369, 15)])
                _gq = [0]

                def qkv_ps():
                    # only mm0/mm1 here: po/sc banks stay free for the
                    # interleaved attention chains
                    t = ("mm0", "mm1")[_gq[0] % 2]
                    _gq[0] += 1
                    return p_ps.tile([P, 512], f32, tag=t, name="gq")

                def emit_attn_qt(qt):
                    for h_ in range(H):
                        kcs, mks, wins = [], [], []
                        for c in range(4):
                            kc = 3 * qt + (c - 1 if lower else c)
                            if 0 <= kc < NTC:
                                kcs.append(kc)
                                mks.append(f"m_{'lo' if lower else 'up'}{c}")
                                wins.append(WINS[c])
                        attn_head_qt(kt_t, qt_t, v_t, h_, qt * NT, NT, kcs,
                                     mks, ot_strips, windows=wins)

                # emit qkv per token tile, with each attention query tile
                # interleaved as soon as its K/V columns exist (in-order
                # engine streams otherwise serialize attention behind the
                # whole projection)
                for nt in range(NQT):
                    o0 = nt * NT
                    for oc in range(EC):
                        ps = qkv_ps()
                        for ec in range(EC):
                            nc.tensor.matmul(
                                ps[:, :NT],
                                wqkv[:, ec, oc * P:(oc + 1) * P],
                                h1[ec][:, o0:o0 + NT],
                                start=(ec == 0), stop=(ec == EC - 1))
                        psum_to_sbuf(qt_t[:, oc, o0:o0 + NT], ps[:, :NT],
                                     i, "qkv", oc, eng="act")
                        ps = qkv_ps()
                        for ec in range(EC):
                            nc.tensor.matmul(
                                ps[:, :NT],
                                wqkv[:, ec, E + oc * P:E + (oc + 1) * P],
                                h1[ec][:, o0:o0 + NT],
                                start=(ec == 0), stop=(ec == EC - 1))
                        psum_to_sbuf(kt_t[:, oc, o0:o0 + NT], ps[:, :NT],
                                     i, "qkv", EC + oc, eng="act")
                    for tck in range(3 * nt, 3 * nt + 3):
                        ps = qkv_ps()
                        for ec in range(EC):
                            nc.tensor.matmul(ps[:],
                                             h1[ec][:, tck * P:(tck + 1) * P],
                                             wqkv[:, ec, 2 * E:3 * E],
                                             start=(ec == 0), stop=(ec == EC - 1))
                        if use_bias:
                            add_vbias(i, ps)
                        act(v_t[:, tck, :, 0:HD],
                            ps[:].rearrange("p (h d) -> p h d", h=H), AF.Copy)
                    aq = nt if lower else nt - 1
                    if aq >= 0:
                        emit_attn_qt(aq)
                if not lower:
                    emit_attn_qt(NQT - 1)
                mark(f"b{i}.projmlp")
                x_new = [p_x.tile([P, T], f32, tag=f"x{s}", name=f"xa{i}_{s}")
                         for s in range(EC)]
                def post_out(oc, o, n, ps, i=i, x_new=x_new, x_cur=x_cur):
                    if use_bias:
                        nc.vector.tensor_scalar(ps[:, :n], ps[:, :n],
                                                bslice(i, "out", oc), None, OP.add)
                    nc.vector.tensor_add(x_new[oc][:, o:o + n],
                                         x_cur[oc][:, o:o + n], ps[:, :n])
                gemm(wout, 0, EC, ot_strips, T, post_out)
                x_cur = mlp(i, x_new)

            mark("out")
            # output
            for s in range(EC):
                nc.sync.dma_start(yT[P * s:P * (s + 1), :], x_cur[s][:])
            _lstack.close()


        for _rep in range(repeat):
            _emit_once()

        # record build-order instruction -> phase map (before scheduling)
        _PHASE_OF.clear()
        names = [ins.name for bb in nc.main_func.blocks for ins in bb.instructions]
        bounds = [n for _, n in _PHASE_MARKS]
        labels = [l for l, _ in _PHASE_MARKS]
        import bisect as _bis
        for idx, nm in enumerate(names):
            j = _bis.bisect_right(bounds, idx) - 1
            _PHASE_OF[nm] = labels[j] if j >= 0 else "pre"

    nc.compile()
    return nc


# ------------------------------------------------------------------
# cached executor (compile once, run many)
# ------------------------------------------------------------------
class _Exec:
    def __init__(self, use_bias: bool):
        import jax
        import concourse.mybir as mybir
        from concourse import bass2jax
        from concourse.bass2jax import install_neuronx_cc_hook, _bass_exec_p
        from jax.sharding import Mesh, PartitionSpec
        from jax.experimental.shard_map import shard_map

        install_neuronx_cc_hook()
        nc = _build_nc(use_bias)
        self.nc = nc

        part_name = (nc.partition_id_tensor.name
                     if nc.partition_id_tensor is not None else None)
        in_names, out_names, out_avals = [], [], []
        self.zero_shapes = []
        for alloc in nc.m.functions[0].allocations:
            if not isinstance(alloc, mybir.MemoryLocationSet):
                continue
            name = alloc.memorylocations[0].name
            if alloc.kind == "ExternalInput":
                if name != part_name:
                    in_names.append(name)
            elif alloc.kind == "ExternalOutput":
                out_names.append(name)
                shape = tuple(alloc.tensor_shape)
                dtype = mybir.dt.np(alloc.dtype)
                out_avals.append(jax.core.ShapedArray(shape, dtype))
                self.zero_shapes.append((shape, dtype))
        n_params = len(in_names)
        all_in = in_names + out_names
        if part_name is not None:
            all_in = all_in + [part_name]
        self.in_names = in_names
        self.out_names = out_names
        n_outs = len(out_names)

        def _body(*args):
            operands = list(args)
            if part_name is not None:
                operands.append(bass2jax.partition_id_tensor())
            outs = _bass_exec_p.bind(
                *operands,
                out_avals=tuple(out_avals),
                in_names=tuple(all_in),
                out_names=tuple(out_names),
                lowering_input_output_aliases=(),
                sim_require_finite=True,
                sim_require_nnan=True,
                nc=nc,
            )
            return tuple(outs)
        self._body = _body

        devices = jax.devices()[:N_CORES]
        mesh = Mesh(np.asarray(devices), ("core",))
        in_specs = (PartitionSpec("core"),) * (n_params + n_outs)
        out_specs = (PartitionSpec("core"),) * n_outs
        donate = tuple(range(n_params, n_params + n_outs))
        self.fn = jax.jit(
            shard_map(_body, mesh=mesh, in_specs=in_specs,
                      out_specs=out_specs, check_rep=False),
            donate_argnums=donate, keep_unused=True)
        self.out_avals = out_avals

    def bench(self, in_maps, iters=10):
        """device-resident-input timing: returns per-iteration seconds"""
        import time
        import jax
        from jax.sharding import Mesh, PartitionSpec, NamedSharding
        if not hasattr(self, "_bench_fn"):
            from jax.experimental.shard_map import shard_map
            devices = jax.devices()[:N_CORES]
            mesh = Mesh(np.asarray(devices), ("core",))
            n_in = len(self.in_names) + len(self.zero_shapes)
            self._bench_fn = jax.jit(
                shard_map(self._body, mesh=mesh,
                          in_specs=(PartitionSpec("core"),) * n_in,
                          out_specs=(PartitionSpec("core"),) * len(self.out_names),
                          check_rep=False),
                keep_unused=True)
            self._bench_sharding = NamedSharding(mesh, PartitionSpec("core"))
        concat_in = [
            np.concatenate([np.asarray(in_maps[c][n]) for c in range(N_CORES)], axis=0)
            for n in self.in_names
        ] + [np.zeros((N_CORES * s[0], *s[1:]), d) for (s, d) in self.zero_shapes]
        import jax
        dev_in = [jax.device_put(a, self._bench_sharding) for a in concat_in]
        out = jax.block_until_ready(self._bench_fn(*dev_in))  # warm/compile
        t0 = time.time()
        for _ in range(iters):
            out = self._bench_fn(*dev_in)
        jax.block_until_ready(out)
        return (time.time() - t0) / iters

    def run(self, in_maps):
        """in_maps: list of 8 dicts name->np.ndarray. returns list of dicts"""
        concat_in = [
            np.concatenate([np.asarray(in_maps[c][n]) for c in range(N_CORES)], axis=0)
            for n in self.in_names
        ]
        concat_zeros = [np.zeros((N_CORES * s[0], *s[1:]), d)
                        for (s, d) in self.zero_shapes]
        outs = self.fn(*concat_in, *concat_zeros)
        import jax
        outs = jax.block_until_ready(outs)
        res = []
        for c in range(N_CORES):
            d = {}
            for idx, n in enumerate(self.out_names):
                shp = self.out_avals[idx].shape
                d[n] = np.asarray(outs[idx]).reshape(N_CORES, *shp)[c]
            res.append(d)
        return res


def _get_exec(use_bias: bool) -> "_Exec":
    key = bool(use_bias)
    if key not in _EXEC_CACHE:
        _EXEC_CACHE[key] = _Exec(key)
    return _EXEC_CACHE[key]


# ------------------------------------------------------------------
# host-side input prep
# ------------------------------------------------------------------
def _band_masks():
    ki = np.arange(P)[:, None]
    qi = np.arange(NT)[None, :]
    out = {}
    for c in range(4):
        # lower band, k-chunk kc = 3t + (c-1):  0 <= (i-k) <= W-1 with
        # i-k = qi - ki + 128*(1-c)
        d = qi - ki + P * (1 - c)
        out[f"m_lo{c}"] = ((d >= 0) & (d <= W_BAND - 1)).astype(bf16)
        # upper band, k-chunk kc = 3t + c:  0 <= (k-i) <= W-1 with
        # k-i = ki - qi + 128*c
        d = ki - qi + P * c
        out[f"m_up{c}"] = ((d >= 0) & (d <= W_BAND - 1)).astype(bf16)
    return out


def kernel(x, ln1_w, ln1_b, ln2_w, ln2_b, qkv_w, qkv_b, out_w, out_b,
           fc1_w, fc1_b, fc2_w, fc2_b):
    x = np.asarray(x, np.float32)
    ln1_w = np.asarray(ln1_w, np.float32); ln1_b = np.asarray(ln1_b, np.float32)
    ln2_w = np.asarray(ln2_w, np.float32); ln2_b = np.asarray(ln2_b, np.float32)
    qkv_w = np.asarray(qkv_w, np.float32); qkv_b = np.asarray(qkv_b, np.float32)
    out_w = np.asarray(out_w, np.float32); out_b = np.asarray(out_b, np.float32)
    fc1_w = np.asarray(fc1_w, np.float32); fc1_b = np.asarray(fc1_b, np.float32)
    fc2_w = np.asarray(fc2_w, np.float32); fc2_b = np.asarray(fc2_b, np.float32)

    # fold LN affine into following projection weights (exact):
    #   h = z*w + b  =>  h @ Wt.T = z @ (W*w).T + b @ W.T
    wq_f, wo_f, w1_f, w2_f = [], [], [], []
    biases = []
    for i in range(3):
        qw = qkv_w[i] * ln1_w[i][None, :]
        qb = qkv_b[i] + qkv_w[i] @ ln1_b[i]
        f1 = fc1_w[i] * ln2_w[i][None, :]
        f1b = fc1_b[i] + fc1_w[i] @ ln2_b[i]
        wq_f.append(qw); w1_f.append(f1)
        wo_f.append(out_w[i]); w2_f.append(fc2_w[i])
        # packed per-feature bias tile [P, 32] (q, k, out, fc1, fc2) + v row
        bias_pack = np.zeros((P, 32), np.float32)
        bias_pack[:, 0:8] = qb[:2 * E].reshape(8, P).T
        bias_pack[:, 8:12] = out_b[i].reshape(4, P).T
        bias_pack[:, 12:28] = f1b.reshape(16, P).T
        bias_pack[:, 28:32] = fc2_b[i].reshape(4, P).T
        biases.append((bias_pack, qb[2 * E:].reshape(1, E).astype(bf16)))
    use_bias = any(np.abs(b).max() > 0 or np.abs(np.asarray(v, np.float32)).max() > 0
                   for b, v in biases)

    ex = _get_exec(use_bias)

    masks = _band_masks()
    # weight tensors, transposed to [EC, P, out] layout
    weights = {}
    for i in range(3):
        weights[f"qkvT{i}"] = np.ascontiguousarray(
            wq_f[i].T.reshape(EC, P, 3 * E)).astype(bf16)
        weights[f"outT{i}"] = np.ascontiguousarray(
            wo_f[i].T.reshape(EC, P, E)).astype(bf16)
        weights[f"fc1T{i}"] = np.ascontiguousarray(
            w1_f[i].T.reshape(EC, P, FF)).astype(bf16)
        weights[f"fc2T{i}"] = np.ascontiguousarray(
            w2_f[i].T.reshape(FFC, P, E)).astype(bf16)
        if use_bias:
            weights[f"bias{i}"] = biases[i][0]
            weights[f"vbias{i}"] = biases[i][1]

    in_maps = []
    for core in range(N_CORES):
        b, c = divmod(core, CHUNKS)
        e0 = EXT_STARTS[c]
        xT_ext = np.ascontiguousarray(x[b, e0:e0 + T, :].T)
        xfT = np.ascontiguousarray(x[b].T).astype(bf16)
        m = {"xT": xT_ext, "xfT": xfT, **weights, **masks}
        in_maps.append(m)

    res = ex.run(in_maps)

    out = np.empty((B, S, E), np.float32)
    for core in range(N_CORES):
        b, c = divmod(core, CHUNKS)
        yT_ = res[core]["yT"]               # [E, T]
        off = OWN_OFF[c]
        out[b, OWN * c:OWN * (c + 1), :] = yT_[:, off:off + OWN].T
    return out



# revision 22
# speedup vs baseline: 1.0827x; 1.0827x over previous
"""Trainium2 Bass kernel for nn_AlternateAttention (3-block transformer:
global attention, lower-band attention, upper-band attention, each with MLP).

Sharding: 8 cores = 2 batches x 4 sequence chunks of 1024 tokens, each core
processing an extended window of 1152 tokens (64-token halo each side) so the
banded blocks need no inter-core communication. Block 0 (global attention)
needs full-sequence K/V; each core recomputes them from the replicated raw
input (LN1+KV projection over the full 4096 tokens of its batch).

Layout: activations live TRANSPOSED in SBUF ([feature, token]) so projections
are natural matmuls (lhsT = W^T chunks). LN stats are computed with ones-
matmul partition reductions; per-token mean/rstd are broadcast with K=1
matmuls. Softmax is computed without max subtraction (scores are O(1) by
construction), in the scores^T layout, with row-of-ones appended to V to get
the normalizer for free.
"""
import contextlib
import numpy as np
import ml_dtypes

# ---- problem constants (hardcoded per contract) ----
B, S, E, H, W_BAND, FF = 2, 4096, 512, 8, 16, 2048
HD = E // H                      # 64
N_CORES = 8
CHUNKS = 4                       # sequence chunks per batch
OWN = S // CHUNKS                # 1024
HALO = 64
T = OWN + 2 * HALO               # 1152 = 9*128
P = 128
NT = 384                         # token tile (3 per T)
NQT = T // NT                    # 3
NKC = S // P                     # 32 k-chunks for global attention
NTC = T // P                     # 9 token chunks of ext window
EC = E // P                      # 4 feature strips
FFC = FF // P                    # 16

EXT_STARTS = [max(0, min(OWN * c - HALO, S - T)) for c in range(CHUNKS)]
OWN_OFF = [OWN * c - EXT_STARTS[c] for c in range(CHUNKS)]

_EXEC_CACHE = {}
_PHASE_MARKS = []
_PHASE_OF = {}

bf16 = ml_dtypes.bfloat16
fp8e4 = ml_dtypes.float8_e4m3


def _pack_dr(W):
    """W [out, in] -> DoubleRow-paired lhsT [K//256, P, 2, out] fp8:
    element (e, k, j, m) = W.T[128*(2e+j)+k, m]"""
    Wt = np.ascontiguousarray(W.T)
    K, M = Wt.shape
    return np.ascontiguousarray(
        Wt.reshape(K // 256, 2, P, M).transpose(0, 2, 1, 3)).astype(fp8e4)


def _pack_dr2(W):
    """two-term fp8 packing: [hi pairs | lo pairs] along dim 0"""
    hi = _pack_dr(W)
    hi_f = np.asarray(W.T, np.float32) - hi.transpose(0, 2, 1, 3).reshape(
        W.T.shape).astype(np.float32)
    lo = np.ascontiguousarray(
        hi_f.reshape(-1, 2, P, W.shape[0]).transpose(0, 2, 1, 3)).astype(fp8e4)
    return np.concatenate([hi, lo], axis=0)


# ------------------------------------------------------------------
# device program
# ------------------------------------------------------------------
def _patch_act_tables():
    # The bacc table-load placement maps exp -> 'exp_and_others' and
    # ln -> 'natural_log', thrashing the ACT table RAM (~1.3us per switch,
    # dozens of switches). Restrict the choice to the two sets that cover
    # everything this kernel uses so exp/ln never evict each other.
    import concourse.hw_specs as hw_specs
    import concourse.bacc as bacc_mod
    import concourse.bass_interp as bass_interp
    if getattr(hw_specs, "_aa_patched", False):
        return
    orig = hw_specs.get_activation_tables
    keep = {"natural_log_exp_and_others", "gelu_apprx_tanh_and_others"}

    def _gat(arch):
        tabs = orig(arch)
        return {k: (v if k in keep else set()) for k, v in tabs.items()}

    hw_specs.get_activation_tables = _gat
    bacc_mod.get_activation_tables = _gat
    bass_interp.get_activation_tables = _gat
    hw_specs._aa_patched = True


def _build_nc(use_bias: bool, repeat: int = 1):
    import concourse.bacc as bacc
    import concourse.mybir as mybir
    import concourse.tile as tile

    _patch_act_tables()

    f32 = mybir.dt.float32
    b16 = mybir.dt.bfloat16
    f8 = mybir.dt.float8e4
    DRM = mybir.MatmulPerfMode.DoubleRow
    AF = mybir.ActivationFunctionType
    OP = mybir.AluOpType

    nc = bacc.Bacc("TRN2", target_bir_lowering=False, debug=False,
                   num_devices=N_CORES)
    _PHASE_MARKS.clear()

    def mark(label):
        n = sum(len(b.instructions) for b in nc.main_func.blocks)
        _PHASE_MARKS.append((label, n))

    # ---- dram tensors ----
    xT = nc.dram_tensor("xT", [E, T], f32, kind="ExternalInput")
    xfT = nc.dram_tensor("xfT", [E, S], b16, kind="ExternalInput")
    wq, wo, w1, w2, bias_d = [], [], [], [], []
    for i in range(3):
        wq.append(nc.dram_tensor(f"qkvT{i}", [EC // 2, P, 2, 3 * E], f8, kind="ExternalInput"))
        wo.append(nc.dram_tensor(f"outT{i}", [EC, P, E], b16, kind="ExternalInput"))
        # fc1/fc2 ship two fp8 terms (hi then lo) for accuracy: error-free
        # on the PE side, costs only 2x the (4x faster) DR matmuls
        w1.append(nc.dram_tensor(f"fc1T{i}", [EC, P, 2, FF], f8, kind="ExternalInput"))
        w2.append(nc.dram_tensor(f"fc2T{i}", [FFC, P, 2, E], f8, kind="ExternalInput"))
        if use_bias:
            # packed per-feature biases for transposed-layout outputs:
            # [q(4xP) | k(4xP) | out(4xP) | fc1(16xP) | fc2(4xP)] -> [P, 32]
            bias_d.append(nc.dram_tensor(f"bias{i}", [P, 32], f32, kind="ExternalInput"))
            bias_d.append(nc.dram_tensor(f"vbias{i}", [1, E], b16, kind="ExternalInput"))
    # 0/1 bf16 stripe-mask tiles for banded attention (384-query tiles,
    # 4 relative k-chunks each)
    mask_d = {}
    for bnd in ("lo", "up"):
        for c in range(4):
            nm = f"m_{bnd}{c}"
            mask_d[nm] = nc.dram_tensor(nm, [P, NT], b16, kind="ExternalInput")
    yT = nc.dram_tensor("yT", [E, T], f32, kind="ExternalOutput")

    from concourse.tile import add_dep_helper as _adh

    with tile.TileContext(nc) as tc, contextlib.ExitStack() as ctx:
        pool = lambda name, bufs, **kw: ctx.enter_context(
            tc.tile_pool(name=name, bufs=bufs, **kw))

        # Order ACT instructions across table-set boundaries (exp/ln vs gelu)
        # so the activation-table RAM isn't thrashed (~1.3us per reload).
        # Within a set no ordering is imposed.
        _act_sets = {AF.Gelu_apprx_tanh: 1}
        _prev_set = [None]
        _prev_insts = [[]]
        _cur_insts = [[]]

        def act(out, in_, func, **kw):
            inst = nc.scalar.activation(out, in_, func, **kw)
            if func == AF.Copy:
                return inst
            s = _act_sets.get(func, 0)
            if _prev_set[0] is None:
                _prev_set[0] = s
            if s != _prev_set[0]:
                _prev_insts[0] = _cur_insts[0][-64:]
                _cur_insts[0] = []
                _prev_set[0] = s
            for p in _prev_insts[0]:
                _adh(inst.ins, p.ins, sync=True, reason="act-table batch order")
            _cur_insts[0].append(inst)
            return inst

        # ---- pools live for the whole kernel ----
        p_x = pool("x", 2)           # residual strips f32, tags x0..x3
        p_h = pool("h", 1)           # LN output strips bf16
        p_xb = pool("xb", 2)         # per-nt bf16 casts for stats
        p_sq = pool("sq", 4)         # per-nt squares bf16
        p_qt = pool("qt", 1)         # QT [P, EC, T] bf16
        p_ot = pool("ot", 1)         # attention out strips bf16
        p_wqkv = pool("wqkv", 1)     # qkv weights [P, EC, 3E]
        p_wout = pool("wout", 1)
        p_probs = pool("probs", 4 if not use_bias else 3)   # exp outputs bf16
        p_tmp = pool("tmp", 2)       # f32 [P, NT] temporaries
        p_sm = pool("sm", 2)         # small [1, n] stat vectors
        p_c = pool("const", 1)       # ones, masks, biases
        # PSUM: exactly 8 banks, hand-assigned tags
        p_ps = pool("ps", 1, space="PSUM")
        # sc0-sc2: attention scores (+ LN stat sums share sc0/sc1, rstd
        # broadcast shares sc2 - temporally disjoint from scores)
        # po0/po1: attention output accumulators
        # mm0/mm1: gemm accumulators
        # bc0: mean broadcast / softmax 1/l broadcast

        ones = p_c.tile([P, P], b16)
        nc.vector.memset(ones[:], 1.0)
        # residual stream runs in 64x units (weights pre-scaled by 64 so fp8
        # quantization stays in the normal range); eps scales by 64^2
        eps1 = p_c.tile([1, 1], f32)
        nc.vector.memset(eps1[:], 1e-5 * 4096.0)
        masks = {}
        for nm, d in mask_d.items():
            mt = p_c.tile([P, NT], b16, tag=f"mask_{nm}", name=f"mask_{nm}")
            nc.sync.dma_start(mt[:], d[:])
            masks[nm] = mt
        bias_t, vbias_t = [], []
        if use_bias:
            for i in range(3):
                bt = p_c.tile([P, 32], f32, tag=f"bias{i}", name=f"bias{i}")
                nc.sync.dma_start(bt[:], bias_d[2 * i][:])
                bias_t.append(bt)
                vt = p_c.tile([1, E], b16, tag=f"vbias{i}", name=f"vbias{i}")
                nc.sync.dma_start(vt[:], bias_d[2 * i + 1][:])
                vbias_t.append(vt)

        def bslice(i, group, oc):
            base = {"qkv": 0, "out": 8, "fc1": 12, "fc2": 28}[group]
            return bias_t[i][:, base + oc:base + oc + 1]

        def add_vbias(i, ps):
            # V projection output is in normal layout [token, feat]: bias
            # varies along the free dim -> broadcast with a K=1 matmul.
            vb = p_ps.tile([P, 512], f32, tag="bc0", name="vb")
            nc.tensor.matmul(vb[:], ones[0:1, :], vbias_t[i][:],
                             start=True, stop=True)
            vbs = p_tmp.tile([P, 512], f32, tag="vbs", name="vbs")
            nc.vector.tensor_copy(vbs[:], vb[:])
            nc.vector.tensor_add(ps[:], ps[:], vbs[:])

        def psum_to_sbuf(dst_ap, ps_ap, i, group, oc, eng="dve"):
            """copy matmul accumulator to sbuf, adding bias if enabled"""
            if eng == "act" and not use_bias:
                act(dst_ap, ps_ap, AF.Copy)
            elif use_bias:
                nc.vector.tensor_scalar(dst_ap, ps_ap, bslice(i, group, oc),
                                        None, OP.add)
            else:
                nc.vector.tensor_copy(dst_ap, ps_ap)

        def _emit_once():
            # load residual strips
            xs = []
            for s in range(EC):
                t = p_x.tile([P, T], f32, tag=f"x{s}", name=f"xin{s}")
                nc.sync.dma_start(t[:], xT[P * s:P * (s + 1), :])
                xs.append(t)

            # ---------- layernorm ----------
            def layernorm(x_strips, Tn, in_f32, htag):
                """x_strips: 4 strips [P, Tn] (f32 or bf16) -> 2 paired fp8
                tiles [P, 2, Tn] (strip s at h8[s//2][:, s%2, :]), for
                DoubleRow fp8 projections."""
                h8 = [p_h.tile([P, 2, Tn], f8, tag=f"{htag}{e}", name=f"{htag}{e}")
                      for e in range(EC // 2)]
                step = 512 if Tn % 512 == 0 else NT
                nss = [(k * step, min(step, Tn - k * step))
                       for k in range((Tn + step - 1) // step)]
                for (o, n) in nss:
                    sl = slice(o, o + n)
                    s1 = p_ps.tile([1, 512], f32, tag="sc0", name="s1")
                    s2 = p_ps.tile([1, 512], f32, tag="sc1", name="s2")
                    xb_nts = {}
                    for s in range(EC):
                        if in_f32:
                            xb_nt = p_xb.tile([P, 512], b16, tag=f"xbn{s}",
                                              name="xbn")
                            nc.vector.tensor_copy(xb_nt[:, :n], x_strips[s][:, sl])
                            rhs_x = xb_nt[:, :n]
                            xb_nts[s] = rhs_x
                        else:
                            rhs_x = x_strips[s][:, sl]
                        nc.tensor.matmul(s1[:, :n], ones[:, 0:1], rhs_x,
                                         start=(s == 0), stop=(s == EC - 1))
                        sq_nt = p_sq.tile([P, 512], b16, tag="sqn", name="sqn")
                        nc.vector.tensor_mul(sq_nt[:, :n], rhs_x, rhs_x)
                        nc.tensor.matmul(s2[:, :n], ones[:, 0:1], sq_nt[:, :n],
                                         start=(s == 0), stop=(s == EC - 1))
                    m_b = p_sm.tile([1, 512], b16, tag="m_b", name="m_b")
                    nc.vector.tensor_scalar(m_b[:, :n], s1[:, :n], 1.0 / E, None, OP.mult)
                    stt = p_sm.tile([1, 1024], f32, tag="stt", name="stt")
                    sa, sb = stt[:, 0:n], stt[:, 512:512 + n]
                    nc.vector.tensor_scalar(sa, s2[:, :n], 1.0 / E, None, OP.mult)
                    nc.vector.tensor_mul(sb, m_b[:, :n], m_b[:, :n])
                    nc.vector.tensor_sub(sa, sa, sb)
                    act(sb, sa, AF.Ln, bias=eps1[:])
                    lnv = sb
                    r_b = p_sm.tile([1, 512], b16, tag="r_b", name="r_b")
                    act(r_b[:, :n], lnv, AF.Exp, scale=-0.5)
                    mB = p_ps.tile([P, 512], f32, tag="bc0", name="mB")
                    nc.tensor.matmul(mB[:, :n], ones[0:1, :], m_b[:, :n],
                                     start=True, stop=True)
                    rB = p_ps.tile([P, 512], f32, tag="sc2", name="rB")
                    nc.tensor.matmul(rB[:, :n], ones[0:1, :], r_b[:, :n],
                                     start=True, stop=True)
                    mBs = p_tmp.tile([P, 512], b16, tag="mBs", name="mBs")
                    act(mBs[:, :n], mB[:, :n], AF.Copy)
                    rBs = p_tmp.tile([P, 512], b16, tag="rBs", name="rBs")
                    act(rBs[:, :n], rB[:, :n], AF.Copy)
                    for s in range(EC):
                        if in_f32:
                            xbs = xb_nts[s]
                        else:
                            xbs = x_strips[s][:, sl]
                        t0 = p_tmp.tile([P, 512], b16, tag="lnt", name="lnt")
                        nc.vector.tensor_sub(t0[:, :n], xbs, mBs[:, :n])
                        nc.vector.tensor_mul(h8[s // 2][:, s % 2, sl],
                                             t0[:, :n], rBs[:, :n])
                return h8

            # ---------- transposed GEMM ----------
            _gm_cycle = [0]
            _sc_cycle = [0]
            _po_cycle = [0]

            def ps_tile(cycle=False):
                t = ("mm0", "mm1", "po0", "po1")[_gm_cycle[0] % 4]
                _gm_cycle[0] += 1
                return p_ps.tile([P, 512], f32, tag=t, name="gps")

            def sc_tile(tags=("sc0", "sc1", "sc2")):
                t = tags[_sc_cycle[0] % len(tags)]
                _sc_cycle[0] += 1
                return p_ps.tile([P, 512], f32, tag=t, name="sps")

            def po_tile(tags=("po0", "po1")):
                t = tags[_po_cycle[0] % len(tags)]
                _po_cycle[0] += 1
                return p_ps.tile([HD + 1, 512], f32, tag=t, name="po")

            def gemm(w_tile, col0, n_oc, x_strips, Tn, post, n_ec=EC):
                """out^T[oc] = sum_ec w_tile[:,ec,col0+oc*P:...].T @ x_strips[ec]"""
                step = 512 if Tn % 512 == 0 else NT
                nss = [(k * step, min(step, Tn - k * step))
                       for k in range((Tn + step - 1) // step)]
                for (o, n) in nss:
                    for oc in range(n_oc):
                        ps = ps_tile()
                        for ec in range(n_ec):
                            nc.tensor.matmul(
                                ps[:, :n],
                                w_tile[:, ec, col0 + oc * P:col0 + (oc + 1) * P],
                                x_strips[ec][:, o:o + n],
                                start=(ec == 0), stop=(ec == n_ec - 1))
                        post(oc, o, n, ps)

            def gemm_dr(w_tile, col0, n_oc, h8, Tn, post, n_pair=EC // 2,
                        ps_fn=None):
                """fp8 DoubleRow gemm: w_tile [P, KC//2, 2, OUT] fp8, h8 a list
                of paired activation tiles [P, 2, Tn] fp8."""
                ps_fn = ps_fn or ps_tile
                step = 512 if Tn % 512 == 0 else NT
                nss = [(k * step, min(step, Tn - k * step))
                       for k in range((Tn + step - 1) // step)]
                for (o, n) in nss:
                    for oc in range(n_oc):
                        ps = ps_fn()
                        for s0 in range(0, n, 256):
                            ns = min(256, n - s0)
                            for e in range(n_pair):
                                nc.tensor.matmul(
                                    ps[:, s0:s0 + ns],
                                    w_tile[:, e, :, col0 + oc * P:col0 + (oc + 1) * P],
                                    h8[e][:, :, o + s0:o + s0 + ns],
                                    start=(e == 0), stop=(e == n_pair - 1),
                                    perf_mode=DRM, skip_group_check=True)
                        post(oc, o, n, ps)

            # ---------- attention core (shared) ----------
            def attn_head_qt(kt_tile, q_tile, v_tile, h_, qt0, qn, kcs, mask_for,
                             ot_strips, windows=None):
                """one (head, query-tile): scores^T -> exp -> (mask) -> AV -> scale"""
                hp, hh = h_ // 2, h_ % 2
                banded = windows is not None
                po = po_tile(("po0", "po1", "sc2") if banded else ("po0", "po1"))
                first = True
                n_kc = len(kcs)
                covered = []  # disjoint sorted [lo, hi) q-ranges with a start=True writer
                for idx, (kc, mk) in enumerate(zip(kcs, mask_for)):
                    qo, qw = windows[idx] if windows is not None else (0, qn)
                    sps = sc_tile(("sc0", "sc1") if banded else ("sc0", "sc1", "sc2"))
                    nc.tensor.matmul(
                        sps[:, :qw],
                        kt_tile[HD * hh:HD * (hh + 1), hp, kc * P:(kc + 1) * P],
                        q_tile[HD * hh:HD * (hh + 1), hp,
                               qt0 + qo:qt0 + qo + qw],
                        start=True, stop=True)
                    pr = p_probs.tile([P, 512], b16, tag="pr", name="pr")
                    act(pr[:, :qw], sps[:, :qw], AF.Exp, scale=0.125 / 4096.0)
                    if mk is not None:
                        prm = p_probs.tile([P, 512], b16, tag="prm", name="prm")
                        nc.vector.tensor_mul(prm[:, :qw], pr[:, :qw],
                                             masks[mk][:, qo:qo + qw])
                        pr = prm
                    if windows is None:
                        nc.tensor.matmul(po[:, :qn], v_tile[:, kc, h_, :],
                                         pr[:, :qn],
                                         start=first, stop=(idx == n_kc - 1))
                    else:
                        # split the AV into start=True parts (first writer of
                        # those q-columns; PSUM has_written is per element) and
                        # accumulate parts over already-written columns
                        parts = []
                        pos = qo
                        for (clo, chi) in covered + [(qo + qw, qo + qw)]:
                            if pos >= qo + qw:
                                break
                            if chi <= pos:
                                continue
                            if clo > pos:
                                parts.append((pos, min(clo, qo + qw), True))
                            if clo < qo + qw:
                                lo = max(clo, pos)
                                hi = min(chi, qo + qw)
                                if lo < hi:
                                    parts.append((lo, hi, False))
                            pos = max(pos, chi)
                        for (lo, hi, is_new) in parts:
                            nc.tensor.matmul(
                                po[:, lo:hi], v_tile[:, kc, h_, :],
                                pr[:, lo - qo:hi - qo],
                                start=is_new, stop=False,
                                skip_group_check=True)
                        covered.append((qo, qo + qw))
                        covered = sorted(covered)
                        merged = []
                        for (lo, hi) in covered:
                            if merged and lo <= merged[-1][1]:
                                merged[-1] = (merged[-1][0], max(hi, merged[-1][1]))
                            else:
                                merged.append((lo, hi))
                        covered = merged
                    first = False
                ou = p_tmp.tile([HD + 1, 512], b16, tag="ou", name="ou")
                with nc.allow_low_precision(reason="bf16 softmax normalizer"):
                    nc.vector.tensor_copy(ou[:, :qn], po[:, :qn])  # frees po bank
                    linv = p_sm.tile([1, 512], b16, tag="linv", name="linv")
                    nc.vector.reciprocal(linv[:, :qn], ou[HD:HD + 1, :qn])
                lB = p_ps.tile([P, 512], f32, tag="bc0", name="lB")
                nc.tensor.matmul(lB[:HD, :qn], ones[0:1, :HD], linv[:, :qn],
                                 start=True, stop=True)
                nc.vector.tensor_mul(
                    ot_strips[hp][HD * hh:HD * (hh + 1), qt0:qt0 + qn],
                    ou[:HD, :qn], lB[:HD, :qn])

            # ==================================================================
            # BLOCK 0: global attention
            # ==================================================================
            wqkv = p_wqkv.tile([P, EC // 2, 2, 3 * E], f8, tag="wqkv", name="wqkv0")
            for e in range(EC // 2):
                nc.sync.dma_start(wqkv[:, e, :, :], wq[0][e])
            wout = p_wout.tile([P, EC, E], b16, tag="wout", name="wout0")
            for ec in range(EC):
                nc.sync.dma_start(wout[:, ec, :], wo[0][ec])

            mark("b0.ln1+q")
            h1 = layernorm(xs, T, True, "h")
            # Q projection (ext window)
            qt_t = p_qt.tile([P, EC, T], b16, tag="qt", name="qt0")
            gemm_dr(wqkv, 0, EC, h1, T,
                    lambda oc, o, n, ps: psum_to_sbuf(qt_t[:, oc, o:o + n], ps[:, :n],
                                                      0, "qkv", oc))

            mark("b0.kv")
            ot0 = p_ot.tile([P, EC, T], b16, tag="ot", name="ot0")
            ot_strips = [ot0[:, s, :] for s in range(EC)]
            with tc.tile_pool(name="kvfull", bufs=1) as p_kv, \
                 tc.tile_pool(name="xpan", bufs=3 if not use_bias else 1) as p_xp:
                ktf = p_kv.tile([P, EC, S], b16, tag="ktf", name="ktf")
                vf = p_kv.tile([P, NKC, H, HD + 1], b16, tag="vf", name="vf")
                nc.vector.memset(vf[:, :, :, HD:HD + 1], 1.0)
                for pan in range(S // 512):
                    xp = p_xp.tile([P, EC, 512], b16, tag="xp", name="xp")
                    for s in range(EC):
                        nc.sync.dma_start(xp[:, s, :],
                                          xfT[P * s:P * (s + 1), 512 * pan:512 * (pan + 1)])
                    hp_ = layernorm([xp[:, s, :] for s in range(EC)], 512, False, "hp")
                    # K^T columns for this panel
                    for oc in range(EC):
                        ps = ps_tile()
                        for s0 in (0, 256):
                            for e in range(EC // 2):
                                nc.tensor.matmul(
                                    ps[:, s0:s0 + 256],
                                    wqkv[:, e, :, E + oc * P:E + (oc + 1) * P],
                                    hp_[e][:, :, s0:s0 + 256],
                                    start=(e == 0), stop=(e == EC // 2 - 1),
                                    perf_mode=DRM, skip_group_check=True)
                        psum_to_sbuf(ktf[:, oc, 512 * pan:512 * (pan + 1)], ps[:],
                                     0, "qkv", EC + oc, eng="act")
                    # V (normal layout) for this panel
                    for tck in range(4):
                        ps = ps_tile()
                        for s0 in (0, 256):
                            for e in range(EC // 2):
                                nc.tensor.matmul(
                                    ps[:, s0:s0 + 256],
                                    hp_[e][:, :, tck * P:(tck + 1) * P],
                                    wqkv[:, e, :, 2 * E + s0:2 * E + s0 + 256],
                                    start=(e == 0), stop=(e == EC // 2 - 1),
                                    perf_mode=DRM, skip_group_check=True)
                        kc = pan * 4 + tck
                        if use_bias:
                            add_vbias(0, ps)
                        act(vf[:, kc, :, 0:HD],
                            ps[:].rearrange("p (h d) -> p h d", h=H), AF.Copy)
                mark("b0.attn")
                # attention (qt outer so out-proj can start per query tile)
                for (q0, qn_) in ((0, 512), (512, 512), (1024, 128)):
                    for h_ in range(H):
                        attn_head_qt(ktf, qt_t, vf, h_, q0, qn_,
                                     list(range(NKC)), [None] * NKC, ot_strips)

            # ---- pools for the post-block0 phases (opened after kvfull frees,
            # closed at end of emission so repeat>1 can reopen) ----
            _lstack = contextlib.ExitStack()
            lpool = lambda name, bufs, **kw: _lstack.enter_context(
                tc.tile_pool(name=name, bufs=bufs, **kw))
            p_kt = lpool("kt", 1)        # KT (banded) [P, EC, T] bf16
            p_v = lpool("v", 1)          # V_ext [P, NTC, H, HD+1] bf16
            p_g = lpool("g", 2 if not use_bias else 1)          # gelu out [P, FFC, NT] bf16
            p_wfc1 = lpool("wfc1", 1)
            p_wfc2 = lpool("wfc2", 1)

            # ---------- MLP (ln2 + fc1 + gelu + fc2 + residual) ----------
            def mlp(i, x_strips):
                h2 = layernorm(x_strips, T, True, "h")
                wf1 = p_wfc1.tile([P, EC, 2, FF], f8, tag="wfc1", name=f"wfc1_{i}")
                for e in range(EC):
                    nc.sync.dma_start(wf1[:, e, :, :], w1[i][e])
                wf2 = p_wfc2.tile([P, FFC, 2, E], f8, tag="wfc2", name=f"wfc2_{i}")
                for e in range(FFC):
                    nc.sync.dma_start(wf2[:, e, :, :], w2[i][e])
                x_new = [p_x.tile([P, T], f32, tag=f"x{s}", name=f"xm{i}_{s}")
                         for s in range(EC)]
                for nt in range(NQT):
                    o0 = nt * NT
                    g = p_g.tile([P, FFC // 2, 2, NT], f8, tag="g", name="g")
                    for fc in range(FFC):
                        ps = ps_tile()
                        for s0 in range(0, NT, 256):
                            ns = min(256, NT - s0)
                            for e in range(EC):
                                nc.tensor.matmul(
                                    ps[:, s0:s0 + ns],
                                    wf1[:, e, :, fc * P:(fc + 1) * P],
                                    h2[e % (EC // 2)][:, :, o0 + s0:o0 + s0 + ns],
                                    start=(e == 0), stop=(e == EC - 1),
                                    perf_mode=DRM, skip_group_check=True)
                        if use_bias:
                            nc.vector.tensor_scalar(ps[:, :NT], ps[:, :NT],
                                                    bslice(i, "fc1", fc), None, OP.add)
                        act(g[:, fc // 2, fc % 2, :], ps[:, :NT], AF.Gelu_apprx_tanh,
                            scale=1.0 / 64.0)
                    for oc in range(EC):
                        ps = ps_tile()
                        for s0 in range(0, NT, 256):
                            ns = min(256, NT - s0)
                            for e in range(FFC):
                                nc.tensor.matmul(
                                    ps[:, s0:s0 + ns],
                                    wf2[:, e, :, oc * P:(oc + 1) * P],
                                    g[:, e % (FFC // 2), :, s0:s0 + ns],
                                    start=(e == 0), stop=(e == FFC - 1),
                                    perf_mode=DRM, skip_group_check=True)
                        if use_bias:
                            nc.vector.tensor_scalar(ps[:, :NT], ps[:, :NT],
                                                    bslice(i, "fc2", oc), None, OP.add)
                        nc.vector.tensor_add(x_new[oc][:, o0:o0 + NT],
                                             x_strips[oc][:, o0:o0 + NT], ps[:, :NT])
                return x_new

            mark("b0.proj+mlp")
            # block 0 out projection + residual + MLP
            x1 = [p_x.tile([P, T], f32, tag=f"x{s}", name=f"x1_{s}")
                  for s in range(EC)]
            def post_out0(oc, o, n, ps):
                if use_bias:
                    nc.vector.tensor_scalar(ps[:, :n], ps[:, :n],
                                            bslice(0, "out", oc), None, OP.add)
                nc.vector.tensor_add(x1[oc][:, o:o + n], xs[oc][:, o:o + n], ps[:, :n])
            gemm(wout, 0, EC, ot_strips, T, post_out0)
            x1 = mlp(0, x1)

            # ==================================================================
            # BLOCKS 1, 2: banded attention
            # ==================================================================
            mark("banded")
            x_cur = x1
            for i in (1, 2):
                lower = (i == 1)
                mark(f"b{i}.ln1qkv")
                wqkv = p_wqkv.tile([P, EC // 2, 2, 3 * E], f8, tag="wqkv", name=f"wqkv{i}")
                for e in range(EC // 2):
                    nc.sync.dma_start(wqkv[:, e, :, :], wq[i][e])
                wout = p_wout.tile([P, EC, E], b16, tag="wout", name=f"wout{i}")
                for ec in range(EC):
                    nc.sync.dma_start(wout[:, ec, :], wo[i][ec])
                h1 = layernorm(x_cur, T, True, "h")
                qt_t = p_qt.tile([P, EC, T], b16, tag="qt", name=f"qt{i}")
                kt_t = p_kt.tile([P, EC, T], b16, tag="kt", name=f"kt{i}")
                v_t = p_v.tile([P, NTC, H, HD + 1], b16, tag="v", name=f"v{i}")
                nc.vector.memset(v_t[:, :, :, HD:HD + 1], 1.0)
                otb = p_ot.tile([P, EC, T], b16, tag="ot", name=f"ot{i}")
                ot_strips = [otb[:, s, :] for s in range(EC)]
                mark(f"b{i}.attn")
                WINS = ([(0, 15), (0, 143), (128, 143), (256, 128)] if lower
                        else [(0, 143), (113, 143), (241, 143), (369, 15)])
                _gq = [0]

                def qkv_ps():
                    # only mm0/mm1 here: po/sc banks stay free for the
                    # interleaved attention chains
                    t = ("mm0", "mm1")[_gq[0] % 2]
                    _gq[0] += 1
                    return p_ps.tile([P, 512], f32, tag=t, name="gq")

                def emit_attn_qt(qt):
                    for h_ in range(H):
                        kcs, mks, wins = [], [], []
                        for c in range(4):
                            kc = 3 * qt + (c - 1 if lower else c)
                            if 0 <= kc < NTC:
                                kcs.append(kc)
                                mks.append(f"m_{'lo' if lower else 'up'}{c}")
                                wins.append(WINS[c])
                        attn_head_qt(kt_t, qt_t, v_t, h_, qt * NT, NT, kcs,
                                     mks, ot_strips, windows=wins)

                # emit qkv per token tile, with each attention query tile
                # interleaved as soon as its K/V columns exist (in-order
                # engine streams otherwise serialize attention behind the
                # whole projection)
                for nt in range(NQT):
                    o0 = nt * NT
                    for oc in range(EC):
                        ps = qkv_ps()
                        for s0 in range(0, NT, 256):
                            ns = min(256, NT - s0)
                            for e in range(EC // 2):
                                nc.tensor.matmul(
                                    ps[:, s0:s0 + ns],
                                    wqkv[:, e, :, oc * P:(oc + 1) * P],
                                    h1[e][:, :, o0 + s0:o0 + s0 + ns],
                                    start=(e == 0), stop=(e == EC // 2 - 1),
                                    perf_mode=DRM, skip_group_check=True)
                        psum_to_sbuf(qt_t[:, oc, o0:o0 + NT], ps[:, :NT],
                                     i, "qkv", oc, eng="act")
                        ps = qkv_ps()
                        for s0 in range(0, NT, 256):
                            ns = min(256, NT - s0)
                            for e in range(EC // 2):
                                nc.tensor.matmul(
                                    ps[:, s0:s0 + ns],
                                    wqkv[:, e, :, E + oc * P:E + (oc + 1) * P],
                                    h1[e][:, :, o0 + s0:o0 + s0 + ns],
                                    start=(e == 0), stop=(e == EC // 2 - 1),
                                    perf_mode=DRM, skip_group_check=True)
                        psum_to_sbuf(kt_t[:, oc, o0:o0 + NT], ps[:, :NT],
                                     i, "qkv", EC + oc, eng="act")
                    for tck in range(3 * nt, 3 * nt + 3):
                        ps = qkv_ps()
                        for s0 in (0, 256):
                            for e in range(EC // 2):
                                nc.tensor.matmul(
                                    ps[:, s0:s0 + 256],
                                    h1[e][:, :, tck * P:(tck + 1) * P],
                                    wqkv[:, e, :, 2 * E + s0:2 * E + s0 + 256],
                                    start=(e == 0), stop=(e == EC // 2 - 1),
                                    perf_mode=DRM, skip_group_check=True)
                        if use_bias:
                            add_vbias(i, ps)
                        act(v_t[:, tck, :, 0:HD],
                            ps[:].rearrange("p (h d) -> p h d", h=H), AF.Copy)
                    aq = nt if lower else nt - 1
                    if aq >= 0:
                        emit_attn_qt(aq)
                if not lower:
                    emit_attn_qt(NQT - 1)
                mark(f"b{i}.projmlp")
                x_new = [p_x.tile([P, T], f32, tag=f"x{s}", name=f"xa{i}_{s}")
                         for s in range(EC)]
                def post_out(oc, o, n, ps, i=i, x_new=x_new, x_cur=x_cur):
                    if use_bias:
                        nc.vector.tensor_scalar(ps[:, :n], ps[:, :n],
                                                bslice(i, "out", oc), None, OP.add)
                    nc.vector.tensor_add(x_new[oc][:, o:o + n],
                                         x_cur[oc][:, o:o + n], ps[:, :n])
                gemm(wout, 0, EC, ot_strips, T, post_out)
                x_cur = mlp(i, x_new)

            mark("out")
            # output
            for s in range(EC):
                nc.sync.dma_start(yT[P * s:P * (s + 1), :], x_cur[s][:])
            _lstack.close()


        for _rep in range(repeat):
            _emit_once()

        # record build-order instruction -> phase map (before scheduling)
        _PHASE_OF.clear()
        names = [ins.name for bb in nc.main_func.blocks for ins in bb.instructions]
        bounds = [n for _, n in _PHASE_MARKS]
        labels = [l for l, _ in _PHASE_MARKS]
        import bisect as _bis
        for idx, nm in enumerate(names):
            j = _bis.bisect_right(bounds, idx) - 1
            _PHASE_OF[nm] = labels[j] if j >= 0 else "pre"

    nc.compile()
    return nc


# ------------------------------------------------------------------
# cached executor (compile once, run many)
# ------------------------------------------------------------------
class _Exec:
    def __init__(self, use_bias: bool):
        import jax
        import concourse.mybir as mybir
        from concourse import bass2jax
        from concourse.bass2jax import install_neuronx_cc_hook, _bass_exec_p
        from jax.sharding import Mesh, PartitionSpec
        from jax.experimental.shard_map import shard_map

        install_neuronx_cc_hook()
        nc = _build_nc(use_bias)
        self.nc = nc

        part_name = (nc.partition_id_tensor.name
                     if nc.partition_id_tensor is not None else None)
        in_names, out_names, out_avals = [], [], []
        self.zero_shapes = []
        for alloc in nc.m.functions[0].allocations:
            if not isinstance(alloc, mybir.MemoryLocationSet):
                continue
            name = alloc.memorylocations[0].name
            if alloc.kind == "ExternalInput":
                if name != part_name:
                    in_names.append(name)
            elif alloc.kind == "ExternalOutput":
                out_names.append(name)
                shape = tuple(alloc.tensor_shape)
                dtype = mybir.dt.np(alloc.dtype)
                out_avals.append(jax.core.ShapedArray(shape, dtype))
                self.zero_shapes.append((shape, dtype))
        n_params = len(in_names)
        all_in = in_names + out_names
        if part_name is not None:
            all_in = all_in + [part_name]
        self.in_names = in_names
        self.out_names = out_names
        n_outs = len(out_names)

        def _body(*args):
            operands = list(args)
            if part_name is not None:
                operands.append(bass2jax.partition_id_tensor())
            outs = _bass_exec_p.bind(
                *operands,
                out_avals=tuple(out_avals),
                in_names=tuple(all_in),
                out_names=tuple(out_names),
                lowering_input_output_aliases=(),
                sim_require_finite=True,
                sim_require_nnan=True,
                nc=nc,
            )
            return tuple(outs)
        self._body = _body

        devices = jax.devices()[:N_CORES]
        mesh = Mesh(np.asarray(devices), ("core",))
        in_specs = (PartitionSpec("core"),) * (n_params + n_outs)
        out_specs = (PartitionSpec("core"),) * n_outs
        donate = tuple(range(n_params, n_params + n_outs))
        self.fn = jax.jit(
            shard_map(_body, mesh=mesh, in_specs=in_specs,
                      out_specs=out_specs, check_rep=False),
            donate_argnums=donate, keep_unused=True)
        self.out_avals = out_avals

    def bench(self, in_maps, iters=10):
        """device-resident-input timing: returns per-iteration seconds"""
        import time
        import jax
        from jax.sharding import Mesh, PartitionSpec, NamedSharding
        if not hasattr(self, "_bench_fn"):
            from jax.experimental.shard_map import shard_map
            devices = jax.devices()[:N_CORES]
            mesh = Mesh(np.asarray(devices), ("core",))
            n_in = len(self.in_names) + len(self.zero_shapes)
            self._bench_fn = jax.jit(
                shard_map(self._body, mesh=mesh,
                          in_specs=(PartitionSpec("core"),) * n_in,
                          out_specs=(PartitionSpec("core"),) * len(self.out_names),
                          check_rep=False),
                keep_unused=True)
            self._bench_sharding = NamedSharding(mesh, PartitionSpec("core"))
        concat_in = [
            np.concatenate([np.asarray(in_maps[c][n]) for c in range(N_CORES)], axis=0)
            for n in self.in_names
        ] + [np.zeros((N_CORES * s[0], *s[1:]), d) for (s, d) in self.zero_shapes]
        import jax
        dev_in = [jax.device_put(a, self._bench_sharding) for a in concat_in]
        out = jax.block_until_ready(self._bench_fn(*dev_in))  # warm/compile
        t0 = time.time()
        for _ in range(iters):
            out = self._bench_fn(*dev_in)
        jax.block_until_ready(out)
        return (time.time() - t0) / iters

    def run(self, in_maps):
        """in_maps: list of 8 dicts name->np.ndarray. returns list of dicts"""
        concat_in = [
            np.concatenate([np.asarray(in_maps[c][n]) for c in range(N_CORES)], axis=0)
            for n in self.in_names
        ]
        concat_zeros = [np.zeros((N_CORES * s[0], *s[1:]), d)
                        for (s, d) in self.zero_shapes]
        outs = self.fn(*concat_in, *concat_zeros)
        import jax
        outs = jax.block_until_ready(outs)
        res = []
        for c in range(N_CORES):
            d = {}
            for idx, n in enumerate(self.out_names):
                shp = self.out_avals[idx].shape
                d[n] = np.asarray(outs[idx]).reshape(N_CORES, *shp)[c]
            res.append(d)
        return res


def _get_exec(use_bias: bool) -> "_Exec":
    key = bool(use_bias)
    if key not in _EXEC_CACHE:
        _EXEC_CACHE[key] = _Exec(key)
    return _EXEC_CACHE[key]


# ------------------------------------------------------------------
# host-side input prep
# ------------------------------------------------------------------
def _band_masks():
    ki = np.arange(P)[:, None]
    qi = np.arange(NT)[None, :]
    out = {}
    for c in range(4):
        # lower band, k-chunk kc = 3t + (c-1):  0 <= (i-k) <= W-1 with
        # i-k = qi - ki + 128*(1-c)
        d = qi - ki + P * (1 - c)
        out[f"m_lo{c}"] = ((d >= 0) & (d <= W_BAND - 1)).astype(bf16)
        # upper band, k-chunk kc = 3t + c:  0 <= (k-i) <= W-1 with
        # k-i = ki - qi + 128*c
        d = ki - qi + P * c
        out[f"m_up{c}"] = ((d >= 0) & (d <= W_BAND - 1)).astype(bf16)
    return out


def kernel(x, ln1_w, ln1_b, ln2_w, ln2_b, qkv_w, qkv_b, out_w, out_b,
           fc1_w, fc1_b, fc2_w, fc2_b):
    x = np.asarray(x, np.float32)
    ln1_w = np.asarray(ln1_w, np.float32); ln1_b = np.asarray(ln1_b, np.float32)
    ln2_w = np.asarray(ln2_w, np.float32); ln2_b = np.asarray(ln2_b, np.float32)
    qkv_w = np.asarray(qkv_w, np.float32); qkv_b = np.asarray(qkv_b, np.float32)
    out_w = np.asarray(out_w, np.float32); out_b = np.asarray(out_b, np.float32)
    fc1_w = np.asarray(fc1_w, np.float32); fc1_b = np.asarray(fc1_b, np.float32)
    fc2_w = np.asarray(fc2_w, np.float32); fc2_b = np.asarray(fc2_b, np.float32)

    # fold LN affine into following projection weights (exact):
    #   h = z*w + b  =>  h @ Wt.T = z @ (W*w).T + b @ W.T
    wq_f, wo_f, w1_f, w2_f = [], [], [], []
    biases = []
    for i in range(3):
        qw = qkv_w[i] * ln1_w[i][None, :]
        qb = qkv_b[i] + qkv_w[i] @ ln1_b[i]
        f1 = fc1_w[i] * ln2_w[i][None, :]
        f1b = fc1_b[i] + fc1_w[i] @ ln2_b[i]
        wq_f.append(qw); w1_f.append(f1)
        wo_f.append(out_w[i]); w2_f.append(fc2_w[i])
        # packed per-feature bias tile [P, 32] (q, k, out, fc1, fc2) + v row
        bias_pack = np.zeros((P, 32), np.float32)
        bias_pack[:, 0:8] = qb[:2 * E].reshape(8, P).T
        bias_pack[:, 8:12] = out_b[i].reshape(4, P).T
        bias_pack[:, 12:28] = f1b.reshape(16, P).T
        bias_pack[:, 28:32] = fc2_b[i].reshape(4, P).T
        biases.append((bias_pack, qb[2 * E:].reshape(1, E).astype(bf16)))
    use_bias = any(np.abs(b).max() > 0 or np.abs(np.asarray(v, np.float32)).max() > 0
                   for b, v in biases)

    ex = _get_exec(use_bias)

    masks = _band_masks()
    # weight tensors, transposed to [EC, P, out] layout
    weights = {}
    for i in range(3):
        weights[f"qkvT{i}"] = _pack_dr(wq_f[i] * 64.0)
        weights[f"outT{i}"] = np.ascontiguousarray(
            wo_f[i].T.reshape(EC, P, E)).astype(bf16)
        weights[f"fc1T{i}"] = _pack_dr2(w1_f[i] * 64.0)
        weights[f"fc2T{i}"] = _pack_dr2(w2_f[i] * 64.0)
        if use_bias:
            weights[f"bias{i}"] = biases[i][0]
            weights[f"vbias{i}"] = biases[i][1]

    x64 = x * 64.0
    in_maps = []
    for core in range(N_CORES):
        b, c = divmod(core, CHUNKS)
        e0 = EXT_STARTS[c]
        xT_ext = np.ascontiguousarray(x64[b, e0:e0 + T, :].T)
        xfT = np.ascontiguousarray(x64[b].T).astype(bf16)
        m = {"xT": xT_ext, "xfT": xfT, **weights, **masks}
        in_maps.append(m)

    res = ex.run(in_maps)

    out = np.empty((B, S, E), np.float32)
    for core in range(N_CORES):
        b, c = divmod(core, CHUNKS)
        yT_ = res[core]["yT"]               # [E, T], in 64x units
        off = OWN_OFF[c]
        out[b, OWN * c:OWN * (c + 1), :] = yT_[:, off:off + OWN].T * (1.0 / 64.0)
    return out

